# revision 6
# baseline (speedup 1.0000x reference)
"""Trainium2 Bass kernel for nn_Decoder_46660524704357.

Reference computation (shapes hardcoded in DEFAULT_CFG):
    B, C, L, D, E, K = 64, 23, 26000, 64, 512, 3
    eos  = eos_emb @ eos_W.T + eos_b          # [B,C,D]
    bin_emb = emb_table[bin_ids]              # [C,L,D]
    a = bin_emb @ Wb.T                        # [C,L,K]   Wb = fc_W[:, :D]
    e = eos @ We.T + fc_b                     # [B,C,K]   We = fc_W[:, D:]
    out = relu(a[None,:,:,:] + e[:,:,None,:]) # [B,C,L,K]

Sharding: split L across the 8 cores (Lc = 3250 each).  Each core:
  - computes the eos projection chain (tiny) to get e[B,C,K] on-device,
  - for each chromosome c and each output tile, runs ONE fused matmul:
        out[p=(b*K+k), l] = sum_d Wsel[d, p] * embT[d, l] + e_row[p] * 1
    where Wsel[d, b*K+k] = fc_W[k, d] (replicated on-device from a [D,K]
    load) and the (D+1)-th contract row of embT is all-ones so the e term
    rides along.  ScalarE/VectorE apply ReLU on the PSUM->SBUF copy (bf16),
    DMA writes a [B, C, K, Lc] bf16 output.  Host upcasts to fp32 and
    re-interleaves K innermost at the end.

All large tensors ride in bf16 (embT read 9.7MB, out write 28.7MB per core
instead of 19.4/57.4 fp32) — the rel-err budget (2e-2) dwarfs bf16's ~0.4%.

Schedule notes (from NTFF traces):
  - et (embT slice) prefetch rides the otherwise-idle sync HWDGE queue and
    is paced by the emb pool's buffer semaphores, so it cannot crowd out
    the critical eosE/selector loads on the gpsimd queue at startup.
  - e_row is produced in 2-chromosome groups interleaved into the main
    loop, so chromosome 0's matmuls are unblocked ~1us after X is ready
    instead of waiting for the whole e computation.
  - PSUM out tiles span 3 banks (1536 cols): 3x fewer ReLU-eviction
    instructions (the +352cyc ACT / DVE per-instruction overhead was ~90us
    of engine time at 512-col granularity).
"""

import numpy as np
import ml_dtypes

BF16 = ml_dtypes.bfloat16

DEFAULT_CFG = dict(B=64, C=23, L=26000, D=64, E=512, K=3, NCORES=8)

_CACHE = {}


def _derived(cfg):
    B, C, L, D, E, K, NCORES = (cfg[k] for k in ("B", "C", "L", "D", "E", "K", "NCORES"))
    d = dict(cfg)
    d["LC"] = L // NCORES
    d["BC"] = B * C
    d["EP"] = min(128, E)              # contract chunk for eos matmul
    assert E % d["EP"] == 0
    d["NQ"] = E // d["EP"]
    d["ROWS"] = K * B                  # output partition rows (b*K + k)
    # partition tiles over ROWS: cut at b boundaries so each tile's DMA rows
    # merge into contiguous [K*LC] runs per b
    tiles = []
    bmax = 128 // K                    # b's per tile
    b0 = 0
    while b0 < B:
        nb = min(bmax, B - b0)
        tiles.append((b0 * K, nb * K, b0, nb))
        b0 += nb
    d["PTILES"] = tiles                # (p_off, p_n, b0, nb)
    # PSUM out tiles: 3 banks (1536 f32 cols) each, filled by <=512-col matmuls
    big = 1536
    d["NF"] = [big] * (d["LC"] // big) + ([d["LC"] % big] if d["LC"] % big else [])
    d["MM"] = 512                      # matmul moving-operand chunk
    d["EGRP"] = 2                      # chromosomes per e_row colsum group
    d["PREF"] = 5                      # emb pool depth (prefetch pacing)
    return d


def _build_nc(cfg=None):
    import concourse.bass as bass  # noqa: F401
    import concourse.mybir as mybir
    import concourse.tile as tile
    from concourse import bacc

    g = _derived(cfg or DEFAULT_CFG)
    B, C, D, K = g["B"], g["C"], g["D"], g["K"]
    LC, BC, EP, NQ, ROWS = g["LC"], g["BC"], g["EP"], g["NQ"], g["ROWS"]
    MM, EGRP = g["MM"], g["EGRP"]
    FCH = min(512, BC)

    f32 = mybir.dt.float32
    bf16 = mybir.dt.bfloat16
    fsel = bf16   # selector matmul operands (embT stream + wsel stationary)
    feos = bf16   # eos projection matmul operands (eosE stream + eosW stationary)

    # Bacc (not plain Bass): its compile() passes split multi-sem waits and
    # move matmul waits to ldweights — required for walrus codegen.
    nc = bacc.Bacc(None)

    embT = nc.declare_dram_parameter("embT", [D + 1, C * LC], fsel, isOutput=False)
    eosE = nc.declare_dram_parameter("eosE", [EP, NQ * BC], feos, isOutput=False)
    eosW = nc.declare_dram_parameter("eosW", [EP, NQ * D], feos, isOutput=False)
    WbT = nc.declare_dram_parameter("WbT", [D, K], fsel, isOutput=False)
    WeT = nc.declare_dram_parameter("WeT", [D, K], f32, isOutput=False)
    eos_b = nc.declare_dram_parameter("eos_b", [D, 1], f32, isOutput=False)
    fc_b = nc.declare_dram_parameter("fc_b", [1, K], f32, isOutput=False)
    out = nc.declare_dram_parameter("out", [B, C, K, LC], bf16, isOutput=True)

    with tile.TileContext(nc) as tc:
        with (
            tc.tile_pool(name="consts", bufs=1) as consts,
            tc.tile_pool(name="setup_sb", bufs=1) as setup_sb,
            tc.tile_pool(name="emb", bufs=g["PREF"]) as emb_pool,
            tc.tile_pool(name="osb", bufs=6) as osb_pool,
            tc.tile_pool(name="ops", bufs=2, space="PSUM") as ops_pool,
            tc.tile_pool(name="eps", bufs=2, space="PSUM") as eps_pool,
        ):
            # ---- setup loads (gpsimd SWDGE queue — nothing else competes
            # there at startup, so the eos chain's data lands first) -------
            eosE_sb = setup_sb.tile([EP, NQ * BC], feos)
            for q in range(NQ):
                nc.gpsimd.dma_start(
                    eosE_sb[:, q * BC:(q + 1) * BC],
                    eosE[:, q * BC:(q + 1) * BC])
            eosW_sb = setup_sb.tile([EP, NQ * D], feos)
            nc.gpsimd.dma_start(eosW_sb[:, :], eosW[:, :])
            WbT_sb = setup_sb.tile([D, K], fsel)
            nc.gpsimd.dma_start(WbT_sb[:, :], WbT[:, :])
            WeT_sb = setup_sb.tile([D, K], f32)
            nc.gpsimd.dma_start(WeT_sb[:, :], WeT[:, :])
            eosb_sb = setup_sb.tile([D, 1], f32)
            nc.gpsimd.dma_start(eosb_sb[:, :], eos_b[:, :])
            fcb_sb = setup_sb.tile([1, K], f32)
            nc.gpsimd.dma_start(fcb_sb[:, :], fc_b[:, :])

            # selector weights: rows 0..D-1 replicated on-device from WbT
            # (saves the 0.55MB wsel load), row D = e_row (written below)
            se = consts.tile([D + 1, C * ROWS], fsel)
            se_w = se[0:D, :].rearrange("d (r k) -> d r k", r=C * B, k=K)
            WbT_g = WbT_sb[:, :].unsqueeze(1).broadcast_to([D, C * B, K])
            nc.vector.tensor_copy(se_w, WbT_g)

            # eosT[d, cb] = sum_E eos_W[d, E] * eos_emb[cb, E]  (+ eos_b)
            # (cb is c-major: cb = c*B + b — matches selector column order)
            eosT_sb = setup_sb.tile([D, BC], f32)
            bc_chunks = [(i, min(FCH, BC - i)) for i in range(0, BC, FCH)]
            for bc0, nbc in bc_chunks:
                eosT_ps = ops_pool.tile([D, nbc], f32, tag="out_ps")
                for q in range(NQ):
                    nc.tensor.matmul(
                        eosT_ps[:, :],
                        lhsT=eosW_sb[:, q * D:(q + 1) * D],
                        rhs=eosE_sb[:, q * BC + bc0: q * BC + bc0 + nbc],
                        start=(q == 0),
                        stop=(q == NQ - 1),
                    )
                nc.scalar.add(eosT_sb[:, bc0:bc0 + nbc], eosT_ps[:, :], eosb_sb[:, 0:1])

            # X[d, (c,b,k)] = eosT[d, c*B+b] * WeT[d, k]  (bf16 so the
            # per-group colsum matmuls stream at 1 col/cycle)
            X = setup_sb.tile([D, C * ROWS], bf16)
            eosT_g = eosT_sb[:, :].rearrange("d (c b) -> d c b", c=C, b=B) \
                .unsqueeze(3).broadcast_to([D, C, B, K])
            We_g = WeT_sb[:, :].unsqueeze(1).unsqueeze(1).broadcast_to([D, C, B, K])
            X_w = X[:, :].rearrange("d (c b k) -> d c b k", c=C, b=B, k=K)
            nc.vector.tensor_mul(X_w, eosT_g, We_g)
            fcb_g = fcb_sb[:, :].unsqueeze(1).unsqueeze(1).broadcast_to([1, C, B, K])
            X0_w = X[0:1, :].rearrange("d (c b k) -> d c b k", c=C, b=B, k=K)
            nc.vector.tensor_add(X0_w, X0_w.copy(), fcb_g)

            ones64 = setup_sb.tile([D, 1], bf16)
            nc.vector.memset(ones64[:, :], 1.0)

            def emit_e_rows(c0):
                """colsum X over d -> se[D, c0*ROWS : (c0+EGRP)*ROWS]."""
                ncols = min(EGRP, C - c0) * ROWS
                e_ps = eps_pool.tile([D + 1, EGRP * ROWS], f32, tag="e_ps")
                nc.tensor.matmul(
                    e_ps[D:D + 1, 0:ncols],
                    lhsT=ones64[:, 0:1],
                    rhs=X[:, c0 * ROWS: c0 * ROWS + ncols],
                    start=True,
                    stop=True,
                )
                nc.scalar.activation(
                    se[D:D + 1, c0 * ROWS: c0 * ROWS + ncols],
                    e_ps[D:D + 1, 0:ncols],
                    mybir.ActivationFunctionType.Copy,
                )

            # ---- main loop ---------------------------------------------
            out_bkl = out.rearrange("b c k l -> c b (k l)")
            # out-DMAs ride gpsimd (SWDGE, 16-way descriptor fan-out) and
            # scalar (HWDGE); et loads ride sync so their pool-semaphore
            # pacing can't block an out trigger.
            out_engines = [nc.gpsimd, nc.gpsimd, nc.scalar]
            n_dma = 0
            ev = 0
            for c in range(C):
                if c % EGRP == 0:
                    emit_e_rows(c)
                et = emb_pool.tile([D + 1, LC], fsel, tag="embT")
                nc.sync.dma_start(et[:, :], embT[:, c * LC:(c + 1) * LC])
                for ti, (p_off, p_n, b0, nb) in enumerate(g["PTILES"]):
                    so = osb_pool.tile([p_n, LC], bf16, tag="out_sb")
                    f0 = 0
                    for nf in g["NF"]:
                        po = ops_pool.tile([p_n, nf], f32, tag="out_ps")
                        for m0 in range(0, nf, MM):
                            mn = min(MM, nf - m0)
                            nc.tensor.matmul(
                                po[:, m0:m0 + mn],
                                lhsT=se[:, c * ROWS + p_off: c * ROWS + p_off + p_n],
                                rhs=et[:, f0 + m0: f0 + m0 + mn],
                                start=True,
                                stop=True,
                            )
                        # ReLU on PSUM->SBUF copy; alternate engines
                        if ev % 2 == 0:
                            nc.scalar.activation(
                                so[:, f0:f0 + nf], po[:, :],
                                mybir.ActivationFunctionType.Relu,
                            )
                        else:
                            nc.vector.tensor_scalar_max(so[:, f0:f0 + nf], po[:, :], 0.0)
                        ev += 1
                        f0 += nf
                    out_engines[n_dma % len(out_engines)].dma_start(
                        out_bkl[c, b0:b0 + nb, :], so[:, :]
                    )
                    n_dma += 1
    nc.finalize()
    return nc


def _host_prep(eos_emb, bin_ids, emb_table, eos_W, eos_b, fc_W, fc_b, cfg=None):
    """Build the per-core input maps."""
    g = _derived(cfg or DEFAULT_CFG)
    B, C, L, D, E, K = g["B"], g["C"], g["L"], g["D"], g["E"], g["K"]
    NCORES, LC, BC, EP, NQ = g["NCORES"], g["LC"], g["BC"], g["EP"], g["NQ"]

    eos_emb = np.ascontiguousarray(eos_emb, dtype=np.float32)
    emb_table = np.ascontiguousarray(emb_table, dtype=np.float32)
    bin_ids = np.asarray(bin_ids)

    # gather (identity when bin_ids == arange, which is the spec'd fill)
    V = C * L
    flat_ids = bin_ids.reshape(-1)
    if flat_ids.shape[0] == V and emb_table.shape[0] == V and \
            flat_ids[0] == 0 and flat_ids[-1] == V - 1 and \
            np.array_equal(flat_ids, np.arange(V, dtype=flat_ids.dtype)):
        bin_emb = emb_table.reshape(C, L, D)
    else:
        bin_emb = emb_table[bin_ids.reshape(C, L)]

    # eosE[ep, q*BC + c*B + b] = eos_emb[b, c, q*EP + ep]   (c-major bc)
    eosE = np.ascontiguousarray(
        eos_emb.transpose(2, 1, 0).reshape(NQ, EP, BC).transpose(1, 0, 2).reshape(EP, NQ * BC)
    ).astype(BF16)
    eosW = np.ascontiguousarray(
        np.asarray(eos_W, np.float32).T.reshape(NQ, EP, D).transpose(1, 0, 2).reshape(EP, NQ * D)
    ).astype(BF16)
    fc_W = np.asarray(fc_W, np.float32)
    WbT = np.ascontiguousarray(fc_W[:, :D].T).astype(BF16)  # [D, K]
    WeT = np.ascontiguousarray(fc_W[:, D:].T)               # [D, K]
    eos_b_in = np.asarray(eos_b, np.float32).reshape(D, 1)
    fc_b_in = np.asarray(fc_b, np.float32).reshape(1, K)

    shared = dict(eosE=eosE, eosW=eosW, WbT=WbT, WeT=WeT, eos_b=eos_b_in, fc_b=fc_b_in)

    in_maps = []
    for i in range(NCORES):
        sl = bin_emb[:, i * LC:(i + 1) * LC, :]          # [C, Lc, D]
        embT_i = np.empty((D + 1, C * LC), BF16)
        embT_i[:D] = sl.transpose(2, 0, 1).reshape(D, C * LC).astype(BF16)
        embT_i[D] = np.float32(1.0)
        in_maps.append({"embT": embT_i, **shared})
    return in_maps


def _assemble(results, cfg=None):
    g = _derived(cfg or DEFAULT_CFG)
    B, C, L, K, NCORES, LC = g["B"], g["C"], g["L"], g["K"], g["NCORES"], g["LC"]
    out = np.empty((B, C, L, K), np.float32)
    for i in range(NCORES):
        r = results[i]["out"]                            # [B, C, K, Lc] bf16
        out[:, :, i * LC:(i + 1) * LC, :] = r.transpose(0, 1, 3, 2)
    return out


def kernel(eos_emb, bin_ids, emb_table, eos_W, eos_b, fc_W, fc_b):
    from concourse.bass_utils import run_bass_kernel_spmd

    if "nc" not in _CACHE:
        _CACHE["nc"] = _build_nc()
    nc = _CACHE["nc"]
    in_maps = _host_prep(eos_emb, bin_ids, emb_table, eos_W, eos_b, fc_W, fc_b)
    res = run_bass_kernel_spmd(nc, in_maps, core_ids=list(range(DEFAULT_CFG["NCORES"])))
    return _assemble(res.results)
